# revision 46
# baseline (speedup 1.0000x reference)
"""GumbelSparseAttention Trainium2 kernel (8-core SPMD, batch+head sharded).

The reference's straight-through gumbel-softmax mask is numerically a hard
one-hot, so softmax over the -inf-masked scores puts probability 1.0 on
exactly one key per (b, h, q). The q@k^T scores, k-projection and softmax
are dead code. The computation reduces to:
    q = query @ Wq.T                  (only this core's 256 head cols)
    logits_h = q_h @ Wg.T
    idx = argmax(logits_h + gumbel_h)     (per (b, h, query-row))
    attn[:, h] = (value @ Wv.T)[idx, h-cols]   (row gather)
    out_partial = attn_cols @ Wo[:, cols].T    (summed across cores on host)

Sharding: core c owns batch b = c//4 and heads {4g..4g+3}, g = c%4, i.e.
feature columns [256g, 256g+256) of that batch. Loading only one batch's
query/value halves the q/v/out DMA vs. head-only sharding.

Precision: the argmax path (q, Wq, Wg, gumbel, logits) stays fp32/f32r --
any rounding there flips argmax winners on near-ties, which is the dominant
error mode. The value path (v, Wv, gather table, Wo, out partial) runs in
bf16: it only perturbs magnitudes by ~0.3%, far under the harness gate, and
halves its DMA traffic.

The makespan is bound by the argmax stream: every (head, row-tile) needs an
elementwise add (logits+gumbel) plus two full DVE passes (Max + MaxIndex,
no DVE fast modes exist for either). The add is split: row-tiles 0-4 add on
Pool (via an Act PSUM->SBUF staging copy; GPSIMD cannot read PSUM), 5-7 add
on DVE, balancing Pool (adds + 32 SWDGE gathers) against DVE.

Hardware findings baked in (cost model accepts all of these, silicon does
not): tensor_tensor_reduce faults at runtime; TensorScalarPtr is invalid on
Pool; GPSIMD ops cannot touch PSUM; indirect-DMA offset APs must be a
single index column per partition (multi-index gathers return garbage);
interleaving two open matmul accumulation groups in one PSUM bank corrupts
the accumulators (q-proj runs ft-outer for this reason).

Scheduling (a DMA's semaphore wait blocks the issuing engine's sequencer,
so placement matters):
  SP   issues only wait-free loads in deadline order: wq, column-sliced qT
       (so the first logits tile waits for ~2.5MB, not 6.5MB), wg, then the
       gumbel stream with value-path loads slotted into its slack; the
       vrows/out writes (which wait on copies) are emitted behind the last
       gumbel DMA where blocking SP is harmless.
  Act  does every PSUM->SBUF copy and the idx' = 4*idx+h index math.
  PE   projections (psO bank), logits (psL banks, nothing else ever
       allocates there, so PE trails DVE by exactly one tile), and all
       transposes, which write into corners of psO tiles that later
       out-projection matmuls overwrite (start=True) -- freeing enough
       PSUM for double-buffered out-projection.
  Pool runs the rt0-4 adds and the per-(rt, head) single-index gathers
       against the DRAM vrows table.
Gather/out for row-tiles 0-3 is emitted between the last argmax tile's
head-slots (their gathers are ready by the time PE drains to them); the
rest trails post-loop, paced purely by data dependencies. MaxIndex outputs
use overlapping 8-wide windows in a shared index tile so successive
row-tiles leave their slot-0 index values packed contiguously per head.
"""

import numpy as np

import concourse.bass as bass
import concourse.bacc as bacc
import concourse.mybir as mybir
import bass_rust
from concourse.tile import TileContext
from concourse.masks import make_identity
from concourse.bass_utils import run_bass_kernel_spmd

B, S, E, H, HD = 2, 1024, 1024, 16, 64
NCORES = 8
HPC = 4                    # heads per core
FC = HPC * HD              # 256 feature cols per core
f32 = mybir.dt.float32
f32r = mybir.dt.float32r
bf16 = mybir.dt.bfloat16
u32 = mybir.dt.uint32
NEG = -3.0e38
BF = mybir.dt.np(bf16)

# device-validation bisect flags
USE_NEG_TRICK = False       # m8 slots 1-7 = -3e38 vs full vector.max pass
USE_CONS_GATHER = False     # one 512-desc gather w/ idx'=4*idx+h vs 4 per-head
USE_PSO_CORNERS = True     # transposes into psO corners vs dedicated psB pool
USE_TTR = False             # fused tensor_tensor_reduce vs add + max
USE_POOL_ADD = True         # logits+gumbel add on Pool engine vs DVE


def _build():
    nc = bacc.Bacc()
    qT = nc.dram_tensor("qT", [E, S], f32r, kind="ExternalInput")
    vT = nc.dram_tensor("vT", [E, S], bf16, kind="ExternalInput")
    wqT = nc.dram_tensor("wqT", [E, FC], f32r, kind="ExternalInput")
    wvT = nc.dram_tensor("wvT", [E, FC], bf16, kind="ExternalInput")
    wgT = nc.dram_tensor("wgT", [HD, S], f32r, kind="ExternalInput")
    woT = nc.dram_tensor("woT", [FC, E], bf16, kind="ExternalInput")
    gum = nc.dram_tensor("gum", [HPC, S, S], f32, kind="ExternalInput")
    out = nc.dram_tensor("out", [S, E], bf16, kind="ExternalOutput")
    vrows = nc.dram_tensor("vrows", [S, FC], bf16)  # v-proj rows, gather table

    with TileContext(nc) as tc:
        with (
            tc.tile_pool(name="const", bufs=1) as const,
            tc.tile_pool(name="qin", bufs=4) as qin,
            tc.tile_pool(name="vin", bufs=4) as vin,
            tc.tile_pool(name="vmid", bufs=2) as vmid,
            tc.tile_pool(name="vrowt", bufs=2) as vrowt,
            tc.tile_pool(name="gumb", bufs=5) as gumb,
            tc.tile_pool(name="ltmp", bufs=3) as ltmp,
            tc.tile_pool(name="lcp", bufs=3) as lcpp,
            tc.tile_pool(name="mx8", bufs=4) as mx8,
            tc.tile_pool(name="ixa", bufs=9) as ixap,
            tc.tile_pool(name="gat", bufs=3) as gat,
            tc.tile_pool(name="att", bufs=4) as att,
            tc.tile_pool(name="osb", bufs=3) as osb,
            tc.tile_pool(name="psL", bufs=2, space="PSUM") as psL,
            tc.tile_pool(name="psO", bufs=2 if USE_PSO_CORNERS else 1,
                         space="PSUM") as psO,
            tc.tile_pool(name="psB", bufs=1 if USE_PSO_CORNERS else 2,
                         space="PSUM") as psB,
        ):
            # ---- constants (argmax-path weights first: SP priority order) ----
            wq_sb = const.tile([128, 8 * FC], f32r, tag="wq")
            nc.sync.dma_start(wq_sb[:].rearrange("p (k f) -> p k f", f=FC),
                              wqT.rearrange("(k p) f -> p k f", p=128))
            # Wg.T duplicated on both partition halves so each head's q slice
            # (base partition 0 / 64) has a same-base rhs. (Loaded later --
            # after q-slice 0 and rt0's gumbel -- to shorten the DVE ramp.)
            wg_sb = const.tile([128, S], f32r, tag="wg")
            q_sb = const.tile([128, 2 * S], f32r, tag="qcols")  # feat-major q
            wv_sb = const.tile([128, 8 * FC], bf16, tag="wv")
            wo_sb = const.tile([128, 2 * E], bf16, tag="wo")
            ident = const.tile([128, 128], bf16, tag="ident")
            make_identity(nc, ident[:])
            idx_all = const.tile([128, 4 * 16], u32, tag="idxall")
            hpat = const.tile([128, HPC], u32, tag="hpat")
            for h in range(HPC):
                nc.gpsimd.memset(hpat[:, h:h + 1], h)
            # m8 pool buffers: slots 1..7 stay -3e38 forever; ttr writes [0]
            for _ in range(4):
                m8i = mx8.tile([128, 8], f32, tag="m8")
                nc.gpsimd.memset(m8i[:], NEG)

            # ---- q proj, column-sliced: one 256-query-row slice per step ----
            def q_load(pr):
                qt_ = qin.tile([128, 8 * 256], f32r, tag="qin")
                nc.sync.dma_start(
                    qt_[:].rearrange("p (k c) -> p k c", c=256),
                    qT[:, pr * 256:(pr + 1) * 256]
                    .rearrange("(k p) c -> p k c", p=128))
                return qt_

            def q_proj(pr, qt_):
                # ft-outer: only one open accumulation group per PSUM bank
                qp = psO.tile([128, 512], f32, tag="ops")
                for ft in range(2):
                    for k in range(8):
                        w = wq_sb[:, k * FC + ft * 128: k * FC + (ft + 1) * 128]
                        nc.tensor.matmul(
                            qp[:, ft * 256:(ft + 1) * 256], lhsT=w,
                            rhs=qt_[:, k * 256:(k + 1) * 256],
                            start=(k == 0), stop=(k == 7))
                for ft in range(2):
                    nc.scalar.copy(
                        q_sb[:, ft * S + pr * 256: ft * S + (pr + 1) * 256],
                        qp[:, ft * 256:(ft + 1) * 256])


            vw_insts = []
            v_tiles = []
            vcTs = {}
            vps = {}

            def v_load_a():
                nc.sync.dma_start(wv_sb[:].rearrange("p (k f) -> p k f", f=FC),
                                  wvT.rearrange("(k p) f -> p k f", p=128))
                for c in range(2):
                    vt_ = vin.tile([128, 2 * S], bf16, tag="vin")
                    nc.sync.dma_start(
                        vt_[:].rearrange("p (k s) -> p k s", s=S),
                        vT[c * 256:(c + 1) * 256, :].rearrange("(k p) s -> p k s", p=128))
                    v_tiles.append(vt_)

            def v_load_b():
                for c in range(2, 4):
                    vt_ = vin.tile([128, 2 * S], bf16, tag="vin")
                    nc.sync.dma_start(
                        vt_[:].rearrange("p (k s) -> p k s", s=S),
                        vT[c * 256:(c + 1) * 256, :].rearrange("(k p) s -> p k s", p=128))
                    v_tiles.append(vt_)
                nc.sync.dma_start(wo_sb[:].rearrange("p (t e) -> p t e", e=E),
                                  woT.rearrange("(t p) e -> p t e", p=128))

            def v_proj_mm(ft, half):
                # accumulate in psO's bank; psL stays exclusive to logits
                if half == 0:
                    vp_ps = psO.tile([128, S], f32, tag="ops")
                    vps[ft] = vp_ps
                ps = vps[ft]
                for c in (0, 1) if half == 0 else (2, 3):
                    for j in range(2):
                        k = 2 * c + j
                        w = wv_sb[:, k * FC + ft * 128: k * FC + (ft + 1) * 128]
                        for rs in range(2):
                            nc.tensor.matmul(
                                ps[:, rs * 512:(rs + 1) * 512], lhsT=w,
                                rhs=v_tiles[c][:, j * S + rs * 512: j * S + (rs + 1) * 512],
                                start=(k == 0), stop=(k == 7))
                if half == 1:
                    vcT = vmid.tile([128, S], bf16, tag="vmid")
                    nc.scalar.copy(vcT[:], ps[:])
                    vcTs[ft] = vcT

            vrts = {}

            def v_finish(ft):
                vcT = vcTs[ft]
                vrt = vrowt.tile([128, 8 * 128], bf16, tag="vrowt")
                if USE_PSO_CORNERS:
                    po = psO.tile([128, 8 * 128], bf16, tag="ops")
                    for t in range(8):
                        nc.tensor.transpose(po[:, t * 128:(t + 1) * 128],
                                            vcT[:, t * 128:(t + 1) * 128], ident[:])
                        nc.scalar.copy(vrt[:, t * 128:(t + 1) * 128],
                                       po[:, t * 128:(t + 1) * 128])
                else:
                    for t in range(8):
                        tp = psB.tile([128, 128], bf16, tag="small")
                        nc.tensor.transpose(tp[:], vcT[:, t * 128:(t + 1) * 128],
                                            ident[:])
                        nc.scalar.copy(vrt[:, t * 128:(t + 1) * 128], tp[:])
                vrts[ft] = vrt

            def v_write(ft):
                # emitted on SP after the last gumbel DMA: its wait on the Act
                # copies blocks nothing that still matters
                wr = nc.sync.dma_start(
                    vrows[:, ft * 128:(ft + 1) * 128]
                    .rearrange("(t p) c -> p t c", p=128),
                    vrts[ft][:].rearrange("p (t c) -> p t c", c=128))
                vw_insts.append(wr)

            # ---- argmax per (row-tile, head); gather/out paced post-loop ----
            ixas = {}
            vrows4 = vrows.rearrange("s (h f) -> (s h) f", f=HD)

            def gum_load(rt):
                gts = []
                for j in range(2):  # 2 heads per gumbel DMA
                    gt = gumb.tile([128, 2 * S], f32, tag="gum")
                    nc.sync.dma_start(
                        gt[:].rearrange("p (h s) -> p h s", s=S),
                        gum[2 * j:2 * j + 2, rt * 128:(rt + 1) * 128, :]
                        .rearrange("h p s -> p h s"))
                    gts.append(gt)
                return gts

            def argmax_tile(rt, gts=None, after_head=None):
                if gts is None:
                    gts = gum_load(rt)
                for h in range(HPC):
                    lps = psL.tile([128, S], f32, tag="big")
                    pb = (h % 2) * HD
                    lhs = q_sb[pb:pb + HD,
                               (h // 2) * S + rt * 128:(h // 2) * S + (rt + 1) * 128]
                    wgh = wg_sb[pb:pb + HD, :]
                    nc.tensor.matmul(lps[:, 0:512], lhsT=lhs, rhs=wgh[:, 0:512],
                                     start=True, stop=True)
                    nc.tensor.matmul(lps[:, 512:1024], lhsT=lhs, rhs=wgh[:, 512:1024],
                                     start=True, stop=True)
                    tmp = ltmp.tile([128, S], f32, tag="ltmp")
                    m8 = mx8.tile([128, 8], f32, tag="m8")
                    if USE_TTR:
                        # one DVE pass: tmp = logits + gumbel, m8[:,0] = row max
                        nc.vector.tensor_tensor_reduce(
                            out=tmp[:], in0=lps[:],
                            in1=gts[h // 2][:, (h % 2) * S:(h % 2 + 1) * S],
                            scale=1.0, scalar=NEG, op0=mybir.AluOpType.add,
                            op1=mybir.AluOpType.max, accum_out=m8[:, 0:1])
                        if not USE_NEG_TRICK:
                            nc.vector.max(out=m8[:], in_=tmp[:])
                    elif USE_POOL_ADD and rt < 5:
                        # GPSIMD cannot read PSUM: stage logits via Act first
                        lcp = lcpp.tile([128, S], f32, tag="lcp")
                        nc.scalar.copy(lcp[:], lps[:])
                        nc.gpsimd.tensor_add(
                            tmp[:], lcp[:],
                            gts[h // 2][:, (h % 2) * S:(h % 2 + 1) * S])
                        nc.vector.max(out=m8[:], in_=tmp[:])
                    else:
                        nc.vector.tensor_add(
                            tmp[:], lps[:],
                            gts[h // 2][:, (h % 2) * S:(h % 2 + 1) * S])
                        nc.vector.max(out=m8[:], in_=tmp[:])
                    # overlapping 8-wide windows: successive rts overwrite
                    # the prior window's garbage slots, leaving the slot-0
                    # index values packed contiguously per head
                    nc.vector.max_index(
                        out=idx_all[:, h * 16 + rt: h * 16 + rt + 8],
                        in_max=m8[:], in_values=tmp[:])
                    if after_head is not None:
                        after_head(h)
                if USE_CONS_GATHER:
                    # idx' = 4*idx + h folds the head's column offset into the
                    # row index of vrows viewed as [S*4, 64]; tiny DVE op right
                    # after the producing MaxIndexes
                    ixa = ixap.tile([128, HPC], u32, tag="ixa")
                    for h in range(HPC):
                        nc.scalar.activation(
                            out=ixa[:, h:h + 1],
                            in_=idx_all[:, (rt * HPC + h) * 8:(rt * HPC + h) * 8 + 1],
                            func=mybir.ActivationFunctionType.Copy,
                            bias=float(h), scale=float(HPC))
                    ixas[rt] = ixa

            def gather_out(rt):
                gtt = gat.tile([128, FC], bf16, tag="gat")
                gt_ = gtt[:]
                for h in range(HPC):
                    g = nc.gpsimd.indirect_dma_start(
                        out=gt_[:, h * HD:(h + 1) * HD], out_offset=None,
                        in_=vrows[:],
                        in_offset=bass.IndirectOffsetOnAxis(
                            ap=idx_all[:, h * 16 + rt: h * 16 + rt + 1], axis=0),
                        element_offset=h * HD)
                    for wr in vw_insts:
                        bass_rust.add_dep_helper(g.ins, wr.ins, True, "vrows RAW")
                ops = psO.tile([128, E], f32, tag="ops")
                ats = []
                for ft in range(2):
                    if USE_PSO_CORNERS:
                        opsb = ops[:].bitcast(bf16)
                        tp = opsb[:, ft * 128:(ft + 1) * 128]
                    else:
                        tpb = psB.tile([128, 128], bf16, tag="small")
                        tp = tpb[:]
                    nc.tensor.transpose(tp, gt_[:, ft * 128:(ft + 1) * 128], ident[:])
                    at_ = att.tile([128, 128], bf16, tag="att")
                    nc.scalar.copy(at_[:], tp)
                    ats.append(at_)
                for ft in range(2):
                    for rs in range(2):
                        nc.tensor.matmul(
                            ops[:, rs * 512:(rs + 1) * 512], lhsT=ats[ft][:],
                            rhs=wo_sb[:, ft * E + rs * 512: ft * E + (rs + 1) * 512],
                            start=(ft == 0), stop=(ft == 1))
                ob = osb.tile([128, E], bf16, tag="osb")
                for rs in range(2):
                    nc.scalar.copy(ob[:, rs * 512:(rs + 1) * 512],
                                   ops[:, rs * 512:(rs + 1) * 512])
                    nc.sync.dma_start(
                        out[rt * 128:(rt + 1) * 128, rs * 512:(rs + 1) * 512],
                        ob[:, rs * 512:(rs + 1) * 512])

            # deadline-ordered emission: argmax(rt) plus just enough side work
            # to fit PE/SP slack without breaking the gumbel stream
            qt0 = q_load(0)
            gts0 = gum_load(0)
            nc.sync.dma_start(wg_sb[0:HD, :], wgT[:])
            nc.sync.dma_start(wg_sb[HD:128, :], wgT[:])
            q_proj(0, qt0)
            for rt in range(8):
                if rt == 7:
                    argmax_tile(rt, after_head=lambda h: gather_out(h))
                else:
                    argmax_tile(rt, gts0 if rt == 0 else None)
                if rt == 1:
                    q_proj(1, q_load(1))
                elif rt == 2:
                    v_load_a()
                    q_proj(2, q_load(2))
                elif rt == 3:
                    q_proj(3, q_load(3))
                    v_proj_mm(0, 0)
                elif rt == 4:
                    v_load_b()
                    v_proj_mm(0, 1)
                    v_proj_mm(1, 0)
                    v_proj_mm(1, 1)
                    v_finish(0)
                elif rt == 5:
                    v_finish(1)
                elif rt == 6:
                    v_write(0)
                    v_write(1)
            for i in range(4, 8):
                gather_out(i)
    nc.compile()
    return nc


_NC = None


def kernel(query, key, value, Wq, bq, Wk, bk, Wv, bv, Wg, bg, Wo, bo, gumbel_noise,
           _trace=False):
    global _NC
    if _NC is None:
        _NC = _build()
    nc = _NC

    Wq = np.asarray(Wq, np.float32); Wv = np.asarray(Wv, np.float32)
    Wg = np.asarray(Wg, np.float32); Wo = np.asarray(Wo, np.float32)
    bq = np.asarray(bq, np.float32); bg = np.asarray(bg, np.float32)
    bv = np.asarray(bv, np.float32); bo = np.asarray(bo, np.float32)
    gn = np.asarray(gumbel_noise, np.float32)
    wgT = np.ascontiguousarray(Wg.T)
    qTs = [np.ascontiguousarray(np.asarray(query[b], np.float32).T) for b in range(B)]
    vTs = [np.ascontiguousarray(np.asarray(value[b], np.float32).T).astype(BF)
           for b in range(B)]

    in_maps = []
    for c in range(NCORES):
        b, g = divmod(c, 4)
        cols = slice(g * FC, (g + 1) * FC)
        gslice = gn[b, g * HPC:(g + 1) * HPC]
        # fold bg and bq's contribution to logits into the gumbel tensor
        add = np.zeros((HPC, S), np.float32)
        for i in range(HPC):
            hh = g * HPC + i
            add[i] = bg + bq[hh * HD:(hh + 1) * HD] @ Wg.T
        if np.any(add):
            gslice = gslice + add[:, None, :]
        in_maps.append({
            "qT": qTs[b], "vT": vTs[b],
            "wqT": np.ascontiguousarray(Wq[cols, :].T),
            "wvT": np.ascontiguousarray(Wv[cols, :].T).astype(BF),
            "wgT": wgT,
            "woT": np.ascontiguousarray(Wo[:, cols].T).astype(BF),
            "gum": np.ascontiguousarray(gslice),
        })

    res = run_bass_kernel_spmd(nc, in_maps, core_ids=list(range(NCORES)), trace=_trace)
    kernel.last_results = res
    kernel.last_exec_ns = res.exec_time_ns

    out = np.zeros((B, S, E), np.float32)
    for c, r in enumerate(res.results):
        out[c // 4] += np.asarray(r["out"], np.float32)
    out += (bv @ Wo.T + bo)[None, None, :]
    return out.astype(np.float32)


kernel.last_results = None
kernel.last_exec_ns = None
